# revision 8
# baseline (speedup 1.0000x reference)
"""Trainium2 Bass kernel for nn_CalibrationNetwork (dense_mlp).

Network (per sample b with judge j = judge_ids[b], per question q):
    z1 = sigmoid([1,x] @ (W1+W1_a[j])[q])        # [6]->[128]
    z2 = sigmoid([1,z1] @ (W2+W2_a[j]))          # [129]->[128]
    out = softmax([1,z2] @ (V+V_a[j])[q])        # [129]->[5]

Strategy (v3 — scheduled for the ACT-engine roofline):
  - Data parallel over 8 cores; judge weights replicated; host groups
    samples by judge with identical per-judge capacities per core, so one
    static SPMD program serves all cores.
  - sigmoid folded into tanh host-side; softmax as exp (no max-sub; logits
    provably small) with the V-bias as a multiplicative exp(bV) plane.
  - Two 4-bank PSUM tiles (slot A/B) alternate between judges, q-pairs per
    bank; tile-granular dependency tracking then matches the intended
    pipeline hazards exactly. Both judges of a pair put their layer-3 psum
    in slot A's banks 2,3 at a fixed tail (cols 442..510) so one paired
    exp serves two judges.
  - Layer-3 chunks are contiguous halves of each segment; the host packs
    each segment's x columns de-interleaved (col c*H+p holds sample 2p+c)
    so each segment's output is a single 3D-AP DMA descriptor.
  - Input DMAs are batched (x in 2 descriptors) on the SP queue; the
    late-needed tensors ride the GpSimd software-DGE queue; output DMAs
    alternate queues.
"""

import sys

import numpy as np

if "/opt/trn_rl_repo" not in sys.path:
    sys.path.insert(0, "/opt/trn_rl_repo")

B, J, Q, O, H1, H2 = 16384, 12, 7, 5, 128, 128
NCORES = 8
CMAX = 220  # per-segment sample cap (2*C must stay below the col-442 L3 tail)
DT_L1 = "bf16"
DT_L2 = "bf16"
DT_L3 = "bf16"
L3COL = 442  # fixed psum col of the layer-3 tail region (70 cols)


def _np_dt(tag):
    if tag == "bf16":
        import ml_dtypes

        return ml_dtypes.bfloat16
    return np.float32


def _fold_weights(W1, W1_a, W2, W2_a, V, V_a):
    """Per-judge weight transforms (all float32, tiny)."""
    f32 = np.float32
    W1c = (W1[None] + W1_a).astype(f32)  # [J,Q,6,H1]
    W1h = (0.5 * W1c).astype(f32)
    W2c = (W2[None] + W2_a).astype(f32)  # [J,129,H2]
    W2m = (0.25 * W2c[:, 1:, :]).astype(f32)  # [J,H1,H2]
    b2 = (0.5 * W2c[:, 0, :] + 0.25 * W2c[:, 1:, :].sum(1)).astype(f32)  # [J,H2]
    Vc = (V[None] + V_a).astype(f32)  # [J,Q,129,O]
    Vm = (0.5 * Vc[:, :, 1:, :]).astype(f32)  # [J,Q,H2,O]
    bV = (Vc[:, :, 0, :] + 0.5 * Vc[:, :, 1:, :].sum(2)).astype(f32)  # [J,Q,O]
    expb = np.exp(bV).astype(f32)

    w1s = np.ascontiguousarray(W1h.transpose(2, 0, 1, 3).reshape(6, J * Q * H1)).astype(_np_dt(DT_L1))
    w2s = np.ascontiguousarray(W2m.transpose(1, 0, 2).reshape(H1, J * H2)).astype(_np_dt(DT_L2))
    b2s = np.ascontiguousarray(b2.T)  # [H2, J]
    vs = np.ascontiguousarray(Vm.transpose(2, 0, 1, 3).reshape(H2, J * Q * O)).astype(_np_dt(DT_L3))
    return w1s, w2s, b2s, vs, expb.reshape(J, Q * O)


def _expand_expb(expb, segs):
    """Per-segment expb plane aligned with the u-tile layout: 70 cols per
    segment = (chunk c in 2) x (q,o in 35), identical for both chunks."""
    cols = [np.tile(expb[j], 2) for j, n0, C in segs]
    flat = np.concatenate(cols) if cols else np.zeros(0, np.float32)
    return np.ascontiguousarray(
        np.broadcast_to(flat.reshape(1, -1), (128, flat.size))
    ).astype(np.float32)


def _plan(judge_ids):
    """Distribute samples: per judge j, split its samples evenly over the 8
    cores and pad each core's share to a common capacity C_j, so every core
    sees identical segment geometry (one compiled program, SPMD)."""
    jid = np.asarray(judge_ids).astype(np.int64)
    order = np.argsort(jid, kind="stable")
    sorted_j = jid[order]
    caps = []
    core_idx = [[] for _ in range(NCORES)]
    for j in range(J):
        lo = np.searchsorted(sorted_j, j, side="left")
        hi = np.searchsorted(sorted_j, j, side="right")
        idx_j = order[lo:hi]
        cnt = hi - lo
        if cnt == 0:
            caps.append(0)
            continue
        cj = -(-cnt // NCORES)  # ceil
        cj = (cj + 3) // 4 * 4  # 4-elem multiple: aligned bf16 slices, even pairs
        caps.append(cj)
        for c in range(NCORES):
            part = idx_j[c::NCORES]
            if len(part) < cj:
                pad_val = part[-1] if len(part) else idx_j[0]
                part = np.concatenate(
                    [part, np.full(cj - len(part), pad_val, dtype=part.dtype)]
                )
            assert len(part) == cj
            core_idx[c].append(part)
    core_idx = [
        np.concatenate(p) if p else np.zeros(0, dtype=np.int64) for p in core_idx
    ]
    ncap = int(sum(caps))
    segs = []
    n0 = 0
    for j in range(J):
        c = caps[j]
        while c > 0:
            s = min(c, CMAX)
            segs.append((j, n0, s))
            n0 += s
            c -= s
    assert n0 == ncap
    return core_idx, segs, ncap


def _col_order(segs, ncap):
    """Device column order: per segment, col c*H+p holds the sample with
    output row 2p+c, so each segment's output is one regular-AP DMA."""
    idx = np.empty(ncap, dtype=np.int64)
    for jj, n0, C in segs:
        rows = np.arange(n0, n0 + C)
        idx[n0 : n0 + C] = np.concatenate([rows[0::2], rows[1::2]])
    return idx


def _pack_inputs(x, core_idx, segs, ncap, w1s, w2s, b2s, vs, expbs):
    col = _col_order(segs, ncap)
    in_maps = []
    for c in range(NCORES):
        xs = x[core_idx[c][col]]  # [ncap, Q, O] in device column order
        xb = np.empty((6, Q, ncap), dtype=np.float32)
        xb[0, :, :] = 1.0
        xb[1:, :, :] = xs.transpose(2, 1, 0)
        xb = np.ascontiguousarray(xb.astype(_np_dt(DT_L1)))
        in_maps.append(
            {"xb": xb, "w1s": w1s, "w2s": w2s, "b2s": b2s, "vs": vs, "expbs": expbs}
        )
    return in_maps


def _build_program(ncap, segs, reps=1):
    import concourse.bass as bass  # noqa: F401
    import concourse.tile as tile
    from concourse import bacc, mybir

    f32 = mybir.dt.float32
    bf16 = mybir.dt.bfloat16
    mdt = {"f32": f32, "bf16": bf16}
    dt1, dt2, dt3 = mdt[DT_L1], mdt[DT_L2], mdt[DT_L3]
    AF = mybir.ActivationFunctionType
    AX = mybir.AxisListType
    ALU = mybir.AluOpType

    NSEG = len(segs)
    for j, n0, C in segs:
        assert C % 4 == 0 and 2 * C <= L3COL, (j, n0, C)
    CMAXJ = max(C for _, _, C in segs)

    nc = bacc.Bacc("TRN2", target_bir_lowering=False, debug=False, num_devices=NCORES)
    d_xb = nc.dram_tensor("xb", [6, Q * ncap], dt1, kind="ExternalInput")
    d_w1 = nc.dram_tensor("w1s", [6, J * Q * H1], dt1, kind="ExternalInput")
    d_w2 = nc.dram_tensor("w2s", [H1, J * H2], dt2, kind="ExternalInput")
    d_b2 = nc.dram_tensor("b2s", [H2, J], f32, kind="ExternalInput")
    d_v = nc.dram_tensor("vs", [H2, J * Q * O], dt3, kind="ExternalInput")
    d_eb = nc.dram_tensor("expbs", [128, NSEG * 70], f32, kind="ExternalInput")
    d_out = nc.dram_tensor("out", [ncap, Q * O], f32, kind="ExternalOutput")

    with tile.TileContext(nc) as tc:
        with (
            tc.tile_pool(name="singles", bufs=1) as singles,
            tc.tile_pool(name="zp", bufs=2) as zp,
            tc.tile_pool(name="pp", bufs=1, space="PSUM") as pp,
        ):
            # ---- input DMAs: batched x on SP; late-needed weights on GpSimd
            sxb = singles.tile([6, Q * ncap], dt1)
            sw1 = singles.tile([6, J * Q * H1], dt1)
            sw2 = singles.tile([H1, J * H2], dt2)
            sb2 = singles.tile([H2, J], f32)
            sv = singles.tile([H2, J * Q * O], dt3)
            seb = singles.tile([128, NSEG * 70], f32)
            nc.sync.dma_start(out=sxb[:, : 2 * ncap], in_=d_xb.ap()[:, : 2 * ncap])
            nc.sync.dma_start(out=sw1[:], in_=d_w1.ap())
            nc.sync.dma_start(out=sxb[:, 2 * ncap :], in_=d_xb.ap()[:, 2 * ncap :])
            nc.sync.dma_start(out=sw2[:], in_=d_w2.ap())
            nc.sync.dma_start(out=sb2[:], in_=d_b2.ap())
            nc.gpsimd.dma_start(out=sv[:], in_=d_v.ap())
            nc.gpsimd.dma_start(out=seb[:], in_=d_eb.ap())

            u = singles.tile([128, NSEG * 70], f32)
            r = singles.tile([128, NSEG * 14], f32)

            # two 4-bank psum tiles; judge k uses slot k%2 for L1/L2.
            # Layer-3 psum of a (even,odd) judge pair sits in slot A's
            # banks 2 (even) and 3 (odd) at cols L3COL..L3COL+70.
            pA = pp.tile([128, 4, 512], f32, tag="pA", name="pA")
            pB = pp.tile([128, 4, 512], f32, tag="pB", name="pB")
            slots = [pA, pB]

            def mm1(k):
                j, n0, C = segs[k]
                P = pA
                for q in range(Q):
                    nc.tensor.matmul(
                        out=P[:, q // 2, (q % 2) * C : (q % 2) * C + C],
                        lhsT=sw1[:, (j * Q + q) * H1 : (j * Q + q + 1) * H1],
                        rhs=sxb[:, q * ncap + n0 : q * ncap + n0 + C],
                        start=True,
                        stop=True,
                    )

            def act1(k):
                j, n0, C = segs[k]
                P = pA
                z1 = zp.tile([128, 8 * CMAXJ], dt2, tag="z1", name="z1")
                nc.scalar.activation(
                    out=z1[:, : 8 * C].rearrange("p (b c) -> p b c", b=4),
                    in_=P[:, :, : 2 * C],
                    func=AF.Tanh,
                )
                return z1

            def mm2(k, z1):
                j, n0, C = segs[k]
                P = pB
                for b in range(4):
                    w = 2 * C if b < 3 else C
                    nc.tensor.matmul(
                        out=P[:, b, 0:w],
                        lhsT=sw2[:, j * H2 : (j + 1) * H2],
                        rhs=z1[:, b * 2 * C : b * 2 * C + w],
                        start=True,
                        stop=True,
                    )

            def act2(k):
                j, n0, C = segs[k]
                P = pB
                z2 = zp.tile([128, 8 * CMAXJ], dt3, tag="z2", name="z2")
                nc.scalar.activation(
                    out=z2[:, : 8 * C].rearrange("p (b c) -> p b c", b=4),
                    in_=P[:, :, : 2 * C],
                    func=AF.Tanh,
                    bias=sb2[:, j : j + 1],
                )
                return z2

            def mm3(k, z2):
                j, n0, C = segs[k]
                HC = C // 2
                bank = 2 + (k % 2)  # both pair members land in slot A
                for q in range(Q):
                    for c in range(2):
                        nc.tensor.matmul(
                            out=pA[0:HC, bank, L3COL + c * 35 + q * O : L3COL + c * 35 + (q + 1) * O],
                            lhsT=z2[:, q * C + c * HC : q * C + (c + 1) * HC],
                            rhs=sv[:, (j * Q + q) * O : (j * Q + q + 1) * O],
                            start=True,
                            stop=True,
                        )

            def act3(e, npair):
                # exp over the L3 tails of segments e..e+npair-1
                s = e % 2
                nc.scalar.activation(
                    out=u[:, 70 * e : 70 * (e + npair)].rearrange(
                        "p (s c) -> p s c", s=npair
                    ),
                    in_=pA[:, 2 + s : 2 + s + npair, L3COL : L3COL + 70],
                    func=AF.Exp,
                )

            def softmax_dma(e, npair):
                span = 70 * npair
                ug = u[:, 70 * e : 70 * e + span]
                nc.vector.tensor_mul(out=ug, in0=ug, in1=seb[:, 70 * e : 70 * e + span])
                ug3 = ug.rearrange("p (t o) -> p t o", o=O)
                rg = r[:, 14 * e : 14 * e + 14 * npair]
                nc.vector.tensor_reduce(out=rg, in_=ug3, axis=AX.X, op=ALU.add)
                nc.vector.reciprocal(out=rg, in_=rg)
                nc.vector.tensor_mul(
                    out=ug3,
                    in0=ug3,
                    in1=rg.unsqueeze(2).broadcast_to((128, 14 * npair, O)),
                )
                for m in range(e, e + npair):
                    j, n0, C = segs[m]
                    HC = C // 2
                    eng = nc.sync if m % 2 == 0 else nc.gpsimd
                    eng.dma_start(
                        out=d_out.ap()[n0 : n0 + C, :].rearrange(
                            "(p c) e -> p c e", c=2
                        ),
                        in_=u[0:HC, 70 * m : 70 * m + 70].rearrange(
                            "p (c e) -> p c e", c=2
                        ),
                    )

            # ---- software-pipelined main loop ----
            z1s = {}
            z2s = {}
            for k in range(NSEG + 1):
                if k < NSEG:
                    mm1(k)
                    z1s[k] = act1(k)
                if k >= 1:
                    mm2(k - 1, z1s.pop(k - 1))
                    z2s[k - 1] = act2(k - 1)
                    mm3(k - 1, z2s.pop(k - 1))
                    if (k - 1) % 2 == 1:
                        act3(k - 2, 2)
                        softmax_dma(k - 2, 2)
                    elif k == NSEG:
                        act3(k - 1, 1)
                        softmax_dma(k - 1, 1)

    nc.compile()
    return nc


def kernel(x, judge_ids, W1, W1_a, W2, W2_a, V, V_a):
    from concourse import bass_utils

    x = np.ascontiguousarray(np.asarray(x), dtype=np.float32)
    jid = np.asarray(judge_ids)
    w1s, w2s, b2s, vs, expb = _fold_weights(
        np.asarray(W1, np.float32),
        np.asarray(W1_a, np.float32),
        np.asarray(W2, np.float32),
        np.asarray(W2_a, np.float32),
        np.asarray(V, np.float32),
        np.asarray(V_a, np.float32),
    )
    core_idx, segs, ncap = _plan(jid)
    expbs = _expand_expb(expb, segs)

    nc = _build_program(ncap, segs)
    in_maps = _pack_inputs(x, core_idx, segs, ncap, w1s, w2s, b2s, vs, expbs)
    res = bass_utils.run_bass_kernel_spmd(nc, in_maps, core_ids=list(range(NCORES)))

    out_full = np.empty((x.shape[0], Q, O), dtype=np.float32)
    for c in range(NCORES):
        out_full[core_idx[c]] = res.results[c]["out"].reshape(ncap, Q, O)
    return out_full


# revision 10
# speedup vs baseline: 1.2715x; 1.2715x over previous
"""Trainium2 Bass kernel for nn_CalibrationNetwork (dense_mlp).

Network (per sample b with judge j = judge_ids[b], per question q):
    z1 = sigmoid([1,x] @ (W1+W1_a[j])[q])        # [6]->[128]
    z2 = sigmoid([1,z1] @ (W2+W2_a[j]))          # [129]->[128]
    out = softmax([1,z2] @ (V+V_a[j])[q])        # [129]->[5]

Strategy (v3 — scheduled for the ACT-engine roofline):
  - Data parallel over 8 cores; judge weights replicated; host groups
    samples by judge with identical per-judge capacities per core, so one
    static SPMD program serves all cores.
  - sigmoid folded into tanh host-side; softmax as exp (no max-sub; logits
    provably small) with the V-bias as a multiplicative exp(bV) plane.
  - PSUM split: L1 always in a 4-bank tile (q-pairs per bank), L2 in a
    3-bank tile (flat 402-col packing), and layer-3 psum in its own bank
    rotating 3 pair positions. Separate tiles keep the tile-granular
    dependency tracking from chaining the paired exp into the next
    judge's matmuls (the serializing cycle of earlier revisions).
  - Layer-3 chunks are contiguous halves of each segment; the host packs
    each segment's x columns de-interleaved (col c*H+p holds sample 2p+c)
    so each segment's output is a single 3D-AP DMA descriptor.
  - Input DMAs spread across the SP, ACT and GpSimd DGE queues; x per
    question and W1 in per-2-judge chunks so the partition-narrow
    transfers parallelize across DMA rings.
  - Input DMAs are batched (x in 2 descriptors) on the SP queue; the
    late-needed tensors ride the GpSimd software-DGE queue; output DMAs
    alternate queues.
"""

import sys

import numpy as np

if "/opt/trn_rl_repo" not in sys.path:
    sys.path.insert(0, "/opt/trn_rl_repo")

B, J, Q, O, H1, H2 = 16384, 12, 7, 5, 128, 128
NCORES = 8
CMAX = 220  # per-segment sample cap (2*C must stay below the col-442 L3 tail)
DT_L1 = "bf16"
DT_L2 = "bf16"
DT_L3 = "bf16"
L3COL = 442  # fixed psum col of the layer-3 tail region (70 cols)


def _np_dt(tag):
    if tag == "bf16":
        import ml_dtypes

        return ml_dtypes.bfloat16
    return np.float32


def _fold_weights(W1, W1_a, W2, W2_a, V, V_a):
    """Per-judge weight transforms (all float32, tiny)."""
    f32 = np.float32
    W1c = (W1[None] + W1_a).astype(f32)  # [J,Q,6,H1]
    W1h = (0.5 * W1c).astype(f32)
    W2c = (W2[None] + W2_a).astype(f32)  # [J,129,H2]
    W2m = (0.25 * W2c[:, 1:, :]).astype(f32)  # [J,H1,H2]
    b2 = (0.5 * W2c[:, 0, :] + 0.25 * W2c[:, 1:, :].sum(1)).astype(f32)  # [J,H2]
    Vc = (V[None] + V_a).astype(f32)  # [J,Q,129,O]
    Vm = (0.5 * Vc[:, :, 1:, :]).astype(f32)  # [J,Q,H2,O]
    bV = (Vc[:, :, 0, :] + 0.5 * Vc[:, :, 1:, :].sum(2)).astype(f32)  # [J,Q,O]
    expb = np.exp(bV).astype(f32)

    w1s = np.ascontiguousarray(W1h.transpose(2, 0, 1, 3).reshape(6, J * Q * H1)).astype(_np_dt(DT_L1))
    w2s = np.ascontiguousarray(W2m.transpose(1, 0, 2).reshape(H1, J * H2)).astype(_np_dt(DT_L2))
    b2s = np.ascontiguousarray(b2.T)  # [H2, J]
    vs = np.ascontiguousarray(Vm.transpose(2, 0, 1, 3).reshape(H2, J * Q * O)).astype(_np_dt(DT_L3))
    return w1s, w2s, b2s, vs, expb.reshape(J, Q * O)


def _expand_expb(expb, segs):
    """Per-segment expb plane aligned with the u-tile layout: 70 cols per
    segment = (chunk c in 2) x (q,o in 35), identical for both chunks."""
    cols = [np.tile(expb[j], 2) for j, n0, C in segs]
    flat = np.concatenate(cols) if cols else np.zeros(0, np.float32)
    return np.ascontiguousarray(
        np.broadcast_to(flat.reshape(1, -1), (128, flat.size))
    ).astype(np.float32)


def _plan(judge_ids):
    """Distribute samples: per judge j, split its samples evenly over the 8
    cores and pad each core's share to a common capacity C_j, so every core
    sees identical segment geometry (one compiled program, SPMD)."""
    jid = np.asarray(judge_ids).astype(np.int64)
    order = np.argsort(jid, kind="stable")
    sorted_j = jid[order]
    caps = []
    core_idx = [[] for _ in range(NCORES)]
    for j in range(J):
        lo = np.searchsorted(sorted_j, j, side="left")
        hi = np.searchsorted(sorted_j, j, side="right")
        idx_j = order[lo:hi]
        cnt = hi - lo
        if cnt == 0:
            caps.append(0)
            continue
        cj = -(-cnt // NCORES)  # ceil
        cj = (cj + 3) // 4 * 4  # 4-elem multiple: aligned bf16 slices, even pairs
        caps.append(cj)
        for c in range(NCORES):
            part = idx_j[c::NCORES]
            if len(part) < cj:
                pad_val = part[-1] if len(part) else idx_j[0]
                part = np.concatenate(
                    [part, np.full(cj - len(part), pad_val, dtype=part.dtype)]
                )
            assert len(part) == cj
            core_idx[c].append(part)
    core_idx = [
        np.concatenate(p) if p else np.zeros(0, dtype=np.int64) for p in core_idx
    ]
    ncap = int(sum(caps))
    segs = []
    n0 = 0
    for j in range(J):
        c = caps[j]
        while c > 0:
            s = min(c, CMAX)
            segs.append((j, n0, s))
            n0 += s
            c -= s
    assert n0 == ncap
    return core_idx, segs, ncap


def _col_order(segs, ncap):
    """Device column order: per segment, col c*H+p holds the sample with
    output row 2p+c, so each segment's output is one regular-AP DMA."""
    idx = np.empty(ncap, dtype=np.int64)
    for jj, n0, C in segs:
        rows = np.arange(n0, n0 + C)
        idx[n0 : n0 + C] = np.concatenate([rows[0::2], rows[1::2]])
    return idx


def _pack_inputs(x, core_idx, segs, ncap, w1s, w2s, b2s, vs, expbs):
    col = _col_order(segs, ncap)
    in_maps = []
    for c in range(NCORES):
        xs = x[core_idx[c][col]]  # [ncap, Q, O] in device column order
        xb = np.empty((Q, 6, ncap), dtype=np.float32)
        xb[:, 0, :] = 1.0
        xb[:, 1:, :] = xs.transpose(1, 2, 0)
        xb = np.ascontiguousarray(xb.astype(_np_dt(DT_L1)))
        in_maps.append(
            {"xb": xb, "w1s": w1s, "w2s": w2s, "b2s": b2s, "vs": vs, "expbs": expbs}
        )
    return in_maps


def _build_program(ncap, segs, reps=1):
    import concourse.bass as bass  # noqa: F401
    import concourse.tile as tile
    from concourse import bacc, mybir

    f32 = mybir.dt.float32
    bf16 = mybir.dt.bfloat16
    mdt = {"f32": f32, "bf16": bf16}
    dt1, dt2, dt3 = mdt[DT_L1], mdt[DT_L2], mdt[DT_L3]
    AF = mybir.ActivationFunctionType
    AX = mybir.AxisListType
    ALU = mybir.AluOpType

    NSEG = len(segs)
    for j, n0, C in segs:
        assert C % 4 == 0 and 2 * C <= L3COL, (j, n0, C)
    CMAXJ = max(C for _, _, C in segs)

    nc = bacc.Bacc("TRN2", target_bir_lowering=False, debug=False, num_devices=NCORES)
    d_xb = nc.dram_tensor("xb", [Q, 6, ncap], dt1, kind="ExternalInput")
    d_w1 = nc.dram_tensor("w1s", [6, J * Q * H1], dt1, kind="ExternalInput")
    d_w2 = nc.dram_tensor("w2s", [H1, J * H2], dt2, kind="ExternalInput")
    d_b2 = nc.dram_tensor("b2s", [H2, J], f32, kind="ExternalInput")
    d_v = nc.dram_tensor("vs", [H2, J * Q * O], dt3, kind="ExternalInput")
    d_eb = nc.dram_tensor("expbs", [128, NSEG * 70], f32, kind="ExternalInput")
    d_out = nc.dram_tensor("out", [ncap, Q * O], f32, kind="ExternalOutput")

    with tile.TileContext(nc) as tc:
        with (
            tc.tile_pool(name="singles", bufs=1) as singles,
            tc.tile_pool(name="zp", bufs=2) as zp,
            tc.tile_pool(name="pp", bufs=1, space="PSUM") as pp,
        ):
            # ---- input DMAs across SP/ACT/GpSimd queues; x per question,
            # W1 in per-2-judge chunks so transfers parallelize over rings
            sxq = [
                singles.tile([6, ncap], dt1, tag=f"xq{q}", name=f"sxq{q}")
                for q in range(Q)
            ]
            sw1 = singles.tile([6, J * Q * H1], dt1)
            sw2 = singles.tile([H1, J * H2], dt2)
            sb2 = singles.tile([H2, J], f32)
            sv = singles.tile([H2, J * Q * O], dt3)
            seb = singles.tile([128, NSEG * 70], f32)
            W1CH = 2 * Q * H1  # w1 chunk: 2 judges
            nc.sync.dma_start(out=sw1[:, :W1CH], in_=d_w1.ap()[:, :W1CH])
            nc.scalar.dma_start(out=sxq[1][:], in_=d_xb.ap()[1])
            nc.gpsimd.dma_start(out=sxq[2][:], in_=d_xb.ap()[2])
            nc.sync.dma_start(out=sxq[0][:], in_=d_xb.ap()[0])
            nc.scalar.dma_start(out=sxq[4][:], in_=d_xb.ap()[4])
            nc.gpsimd.dma_start(out=sxq[5][:], in_=d_xb.ap()[5])
            nc.sync.dma_start(out=sxq[3][:], in_=d_xb.ap()[3])
            nc.scalar.dma_start(out=sw1[:, W1CH : 2 * W1CH], in_=d_w1.ap()[:, W1CH : 2 * W1CH])
            nc.gpsimd.dma_start(out=sxq[6][:], in_=d_xb.ap()[6])
            nc.sync.dma_start(out=sw2[:], in_=d_w2.ap())
            nc.scalar.dma_start(out=sw1[:, 2 * W1CH : 3 * W1CH], in_=d_w1.ap()[:, 2 * W1CH : 3 * W1CH])
            nc.gpsimd.dma_start(out=sw1[:, 3 * W1CH : 4 * W1CH], in_=d_w1.ap()[:, 3 * W1CH : 4 * W1CH])
            nc.sync.dma_start(out=sb2[:], in_=d_b2.ap())
            nc.scalar.dma_start(out=sw1[:, 4 * W1CH : 5 * W1CH], in_=d_w1.ap()[:, 4 * W1CH : 5 * W1CH])
            nc.gpsimd.dma_start(out=sw1[:, 5 * W1CH : 6 * W1CH], in_=d_w1.ap()[:, 5 * W1CH : 6 * W1CH])
            nc.sync.dma_start(out=seb[:], in_=d_eb.ap())
            nc.gpsimd.dma_start(out=sv[:], in_=d_v.ap())

            u = singles.tile([128, NSEG * 70], f32)
            r = singles.tile([128, NSEG * 14], f32)

            # two 4-bank psum tiles; judge k uses slot k%2 for L1/L2.
            # Layer-3 psum of a (even,odd) judge pair sits in slot A's
            # banks 2 (even) and 3 (odd) at cols L3COL..L3COL+70.
            pA = pp.tile([128, 4, 512], f32, tag="pA", name="pA")
            pB = pp.tile([128, 3, 512], f32, tag="pB", name="pB")
            pL3 = pp.tile([128, 1, 512], f32, tag="pL3", name="pL3")

            def mm1(k):
                j, n0, C = segs[k]
                for q in range(Q):
                    nc.tensor.matmul(
                        out=pA[:, q // 2, (q % 2) * C : (q % 2) * C + C],
                        lhsT=sw1[:, (j * Q + q) * H1 : (j * Q + q + 1) * H1],
                        rhs=sxq[q][:, n0 : n0 + C],
                        start=True,
                        stop=True,
                    )

            def act1(k):
                j, n0, C = segs[k]
                P = pA
                z1 = zp.tile([128, 8 * CMAXJ], dt2, tag="z1", name="z1")
                nc.scalar.activation(
                    out=z1[:, : 8 * C].rearrange("p (b c) -> p b c", b=4),
                    in_=P[:, :, : 2 * C],
                    func=AF.Tanh,
                )
                return z1

            def mm2(k, z1):
                j, n0, C = segs[k]
                w = -(-7 * C // 3)
                for b in range(3):
                    nc.tensor.matmul(
                        out=pB[:, b, 0:w],
                        lhsT=sw2[:, j * H2 : (j + 1) * H2],
                        rhs=z1[:, b * w : (b + 1) * w],
                        start=True,
                        stop=True,
                    )

            def act2(k):
                j, n0, C = segs[k]
                w = -(-7 * C // 3)
                z2 = zp.tile([128, 8 * CMAXJ], dt3, tag="z2", name="z2")
                nc.scalar.activation(
                    out=z2[:, : 3 * w].rearrange("p (b c) -> p b c", b=3),
                    in_=pB[:, :, :w],
                    func=AF.Tanh,
                    bias=sb2[:, j : j + 1],
                )
                return z2

            def mm3(k, z2):
                j, n0, C = segs[k]
                HC = C // 2
                pos = 140 * ((k // 2) % 3) + 70 * (k % 2)
                for q in range(Q):
                    for c in range(2):
                        nc.tensor.matmul(
                            out=pL3[0:HC, 0, pos + c * 35 + q * O : pos + c * 35 + (q + 1) * O],
                            lhsT=z2[:, q * C + c * HC : q * C + (c + 1) * HC],
                            rhs=sv[:, (j * Q + q) * O : (j * Q + q + 1) * O],
                            start=True,
                            stop=True,
                        )

            def act3(e, npair):
                # exp over the L3 psum of segments e..e+npair-1
                pos = 140 * ((e // 2) % 3) + 70 * (e % 2)
                nc.scalar.activation(
                    out=u[:, 70 * e : 70 * (e + npair)],
                    in_=pL3[:, 0, pos : pos + 70 * npair],
                    func=AF.Exp,
                )

            def softmax_dma(e, npair):
                span = 70 * npair
                ug = u[:, 70 * e : 70 * e + span]
                nc.vector.tensor_mul(out=ug, in0=ug, in1=seb[:, 70 * e : 70 * e + span])
                ug3 = ug.rearrange("p (t o) -> p t o", o=O)
                rg = r[:, 14 * e : 14 * e + 14 * npair]
                nc.vector.tensor_reduce(out=rg, in_=ug3, axis=AX.X, op=ALU.add)
                nc.vector.reciprocal(out=rg, in_=rg)
                nc.vector.tensor_mul(
                    out=ug3,
                    in0=ug3,
                    in1=rg.unsqueeze(2).broadcast_to((128, 14 * npair, O)),
                )
                for m in range(e, e + npair):
                    j, n0, C = segs[m]
                    HC = C // 2
                    eng = nc.sync if m % 2 == 0 else nc.gpsimd
                    eng.dma_start(
                        out=d_out.ap()[n0 : n0 + C, :].rearrange(
                            "(p c) e -> p c e", c=2
                        ),
                        in_=u[0:HC, 70 * m : 70 * m + 70].rearrange(
                            "p (c e) -> p c e", c=2
                        ),
                    )

            # ---- software-pipelined main loop ----
            z1s = {}
            z2s = {}
            for k in range(NSEG + 1):
                if k < NSEG:
                    mm1(k)
                    z1s[k] = act1(k)
                if k >= 1:
                    mm2(k - 1, z1s.pop(k - 1))
                    z2s[k - 1] = act2(k - 1)
                    mm3(k - 1, z2s.pop(k - 1))
                    if (k - 1) % 2 == 1:
                        act3(k - 2, 2)
                        softmax_dma(k - 2, 2)
                    elif k == NSEG:
                        act3(k - 1, 1)
                        softmax_dma(k - 1, 1)

    nc.compile()
    return nc


def kernel(x, judge_ids, W1, W1_a, W2, W2_a, V, V_a):
    from concourse import bass_utils

    x = np.ascontiguousarray(np.asarray(x), dtype=np.float32)
    jid = np.asarray(judge_ids)
    w1s, w2s, b2s, vs, expb = _fold_weights(
        np.asarray(W1, np.float32),
        np.asarray(W1_a, np.float32),
        np.asarray(W2, np.float32),
        np.asarray(W2_a, np.float32),
        np.asarray(V, np.float32),
        np.asarray(V_a, np.float32),
    )
    core_idx, segs, ncap = _plan(jid)
    expbs = _expand_expb(expb, segs)

    nc = _build_program(ncap, segs)
    in_maps = _pack_inputs(x, core_idx, segs, ncap, w1s, w2s, b2s, vs, expbs)
    res = bass_utils.run_bass_kernel_spmd(nc, in_maps, core_ids=list(range(NCORES)))

    out_full = np.empty((x.shape[0], Q, O), dtype=np.float32)
    for c in range(NCORES):
        out_full[core_idx[c]] = res.results[c]["out"].reshape(ncap, Q, O)
    return out_full


# revision 11
# speedup vs baseline: 1.5069x; 1.1852x over previous
"""Trainium2 Bass kernel for nn_CalibrationNetwork (dense_mlp).

Network (per sample b with judge j = judge_ids[b], per question q):
    z1 = sigmoid([1,x] @ (W1+W1_a[j])[q])        # [6]->[128]
    z2 = sigmoid([1,z1] @ (W2+W2_a[j]))          # [129]->[128]
    out = softmax([1,z2] @ (V+V_a[j])[q])        # [129]->[5]

Strategy (v3 — scheduled for the ACT-engine roofline):
  - Data parallel over 8 cores; judge weights replicated; host groups
    samples by judge with identical per-judge capacities per core, so one
    static SPMD program serves all cores.
  - sigmoid folded into tanh host-side; softmax as exp (no max-sub; logits
    provably small) with the V-bias as a multiplicative exp(bV) plane.
  - PSUM: two 3-bank slots shared by L1 and L2 (judges alternate slots;
    L2 overwrites L1's banks after the first tanh consumed them), flat
    ~402-col packing with layer-1 matmuls split at bank boundaries; the
    layer-3 psum sits in its own bank rotating 3 pair positions so its
    exp never chains into the next judge's matmuls. Layer-3 weight loads
    (z2 chunks) are padded to 128 columns to trigger fast-weight-load.
  - Back-to-back judges keep the tensor engine busy past the ~3.4us HAM
    window so it upshifts to 2.4 GHz.
  - Layer-3 chunks are contiguous halves of each segment; the host packs
    each segment's x columns de-interleaved (col c*H+p holds sample 2p+c)
    so each segment's output is a single 3D-AP DMA descriptor.
  - Input DMAs spread across the SP, ACT and GpSimd DGE queues; x per
    question and W1 in per-2-judge chunks so the partition-narrow
    transfers parallelize across DMA rings.
  - Input DMAs are batched (x in 2 descriptors) on the SP queue; the
    late-needed tensors ride the GpSimd software-DGE queue; output DMAs
    alternate queues.
"""

import sys

import numpy as np

if "/opt/trn_rl_repo" not in sys.path:
    sys.path.insert(0, "/opt/trn_rl_repo")

B, J, Q, O, H1, H2 = 16384, 12, 7, 5, 128, 128
NCORES = 8
CMAX = 220  # per-segment sample cap (2*C must stay below the col-442 L3 tail)
DT_L1 = "bf16"
DT_L2 = "bf16"
DT_L3 = "bf16"
L3COL = 442  # fixed psum col of the layer-3 tail region (70 cols)


def _np_dt(tag):
    if tag == "bf16":
        import ml_dtypes

        return ml_dtypes.bfloat16
    return np.float32


def _fold_weights(W1, W1_a, W2, W2_a, V, V_a):
    """Per-judge weight transforms (all float32, tiny)."""
    f32 = np.float32
    W1c = (W1[None] + W1_a).astype(f32)  # [J,Q,6,H1]
    W1h = (0.5 * W1c).astype(f32)
    W2c = (W2[None] + W2_a).astype(f32)  # [J,129,H2]
    W2m = (0.25 * W2c[:, 1:, :]).astype(f32)  # [J,H1,H2]
    b2 = (0.5 * W2c[:, 0, :] + 0.25 * W2c[:, 1:, :].sum(1)).astype(f32)  # [J,H2]
    Vc = (V[None] + V_a).astype(f32)  # [J,Q,129,O]
    Vm = (0.5 * Vc[:, :, 1:, :]).astype(f32)  # [J,Q,H2,O]
    bV = (Vc[:, :, 0, :] + 0.5 * Vc[:, :, 1:, :].sum(2)).astype(f32)  # [J,Q,O]
    expb = np.exp(bV).astype(f32)

    w1s = np.ascontiguousarray(W1h.transpose(2, 0, 1, 3).reshape(6, J * Q * H1)).astype(_np_dt(DT_L1))
    w2s = np.ascontiguousarray(W2m.transpose(1, 0, 2).reshape(H1, J * H2)).astype(_np_dt(DT_L2))
    b2s = np.ascontiguousarray(b2.T)  # [H2, J]
    vs = np.ascontiguousarray(Vm.transpose(2, 0, 1, 3).reshape(H2, J * Q * O)).astype(_np_dt(DT_L3))
    return w1s, w2s, b2s, vs, expb.reshape(J, Q * O)


def _expand_expb(expb, segs):
    """Per-segment expb plane aligned with the u-tile layout: 70 cols per
    segment = (chunk c in 2) x (q,o in 35), identical for both chunks."""
    cols = [np.tile(expb[j], 2) for j, n0, C in segs]
    flat = np.concatenate(cols) if cols else np.zeros(0, np.float32)
    return np.ascontiguousarray(
        np.broadcast_to(flat.reshape(1, -1), (128, flat.size))
    ).astype(np.float32)


def _plan(judge_ids):
    """Distribute samples: per judge j, split its samples evenly over the 8
    cores and pad each core's share to a common capacity C_j, so every core
    sees identical segment geometry (one compiled program, SPMD)."""
    jid = np.asarray(judge_ids).astype(np.int64)
    order = np.argsort(jid, kind="stable")
    sorted_j = jid[order]
    caps = []
    core_idx = [[] for _ in range(NCORES)]
    for j in range(J):
        lo = np.searchsorted(sorted_j, j, side="left")
        hi = np.searchsorted(sorted_j, j, side="right")
        idx_j = order[lo:hi]
        cnt = hi - lo
        if cnt == 0:
            caps.append(0)
            continue
        cj = -(-cnt // NCORES)  # ceil
        cj = (cj + 3) // 4 * 4  # 4-elem multiple: aligned bf16 slices, even pairs
        caps.append(cj)
        for c in range(NCORES):
            part = idx_j[c::NCORES]
            if len(part) < cj:
                pad_val = part[-1] if len(part) else idx_j[0]
                part = np.concatenate(
                    [part, np.full(cj - len(part), pad_val, dtype=part.dtype)]
                )
            assert len(part) == cj
            core_idx[c].append(part)
    core_idx = [
        np.concatenate(p) if p else np.zeros(0, dtype=np.int64) for p in core_idx
    ]
    ncap = int(sum(caps))
    segs = []
    n0 = 0
    for j in range(J):
        c = caps[j]
        while c > 0:
            s = min(c, CMAX)
            segs.append((j, n0, s))
            n0 += s
            c -= s
    assert n0 == ncap
    return core_idx, segs, ncap


def _col_order(segs, ncap):
    """Device column order: per segment, col c*H+p holds the sample with
    output row 2p+c, so each segment's output is one regular-AP DMA."""
    idx = np.empty(ncap, dtype=np.int64)
    for jj, n0, C in segs:
        rows = np.arange(n0, n0 + C)
        idx[n0 : n0 + C] = np.concatenate([rows[0::2], rows[1::2]])
    return idx


def _pack_inputs(x, core_idx, segs, ncap, w1s, w2s, b2s, vs, expbs):
    col = _col_order(segs, ncap)
    in_maps = []
    for c in range(NCORES):
        xs = x[core_idx[c][col]]  # [ncap, Q, O] in device column order
        xb = np.empty((Q, 6, ncap), dtype=np.float32)
        xb[:, 0, :] = 1.0
        xb[:, 1:, :] = xs.transpose(1, 2, 0)
        xb = np.ascontiguousarray(xb.astype(_np_dt(DT_L1)))
        in_maps.append(
            {"xb": xb, "w1s": w1s, "w2s": w2s, "b2s": b2s, "vs": vs, "expbs": expbs}
        )
    return in_maps


def _build_program(ncap, segs, reps=1):
    import concourse.bass as bass  # noqa: F401
    import concourse.tile as tile
    from concourse import bacc, mybir

    f32 = mybir.dt.float32
    bf16 = mybir.dt.bfloat16
    mdt = {"f32": f32, "bf16": bf16}
    dt1, dt2, dt3 = mdt[DT_L1], mdt[DT_L2], mdt[DT_L3]
    AF = mybir.ActivationFunctionType
    AX = mybir.AxisListType
    ALU = mybir.AluOpType

    NSEG = len(segs)
    for j, n0, C in segs:
        assert C % 4 == 0 and 2 * C <= L3COL, (j, n0, C)
    CMAXJ = max(C for _, _, C in segs)

    nc = bacc.Bacc("TRN2", target_bir_lowering=False, debug=False, num_devices=NCORES)
    d_xb = nc.dram_tensor("xb", [Q, 6, ncap], dt1, kind="ExternalInput")
    d_w1 = nc.dram_tensor("w1s", [6, J * Q * H1], dt1, kind="ExternalInput")
    d_w2 = nc.dram_tensor("w2s", [H1, J * H2], dt2, kind="ExternalInput")
    d_b2 = nc.dram_tensor("b2s", [H2, J], f32, kind="ExternalInput")
    d_v = nc.dram_tensor("vs", [H2, J * Q * O], dt3, kind="ExternalInput")
    d_eb = nc.dram_tensor("expbs", [128, NSEG * 70], f32, kind="ExternalInput")
    d_out = nc.dram_tensor("out", [ncap, Q * O], f32, kind="ExternalOutput")

    with tile.TileContext(nc) as tc:
        with (
            tc.tile_pool(name="singles", bufs=1) as singles,
            tc.tile_pool(name="zp", bufs=2) as zp,
            tc.tile_pool(name="pp", bufs=1, space="PSUM") as pp,
        ):
            # ---- input DMAs across SP/ACT/GpSimd queues; x per question,
            # W1 in per-2-judge chunks so transfers parallelize over rings
            sxq = [
                singles.tile([6, ncap], dt1, tag=f"xq{q}", name=f"sxq{q}")
                for q in range(Q)
            ]
            sw1 = singles.tile([6, J * Q * H1], dt1)
            sw2 = singles.tile([H1, J * H2], dt2)
            sb2 = singles.tile([H2, J], f32)
            sv = singles.tile([H2, J * Q * O], dt3)
            seb = singles.tile([128, NSEG * 70], f32)
            W1CH = 2 * Q * H1  # w1 chunk: 2 judges
            nc.sync.dma_start(out=sw1[:, :W1CH], in_=d_w1.ap()[:, :W1CH])
            nc.scalar.dma_start(out=sxq[1][:], in_=d_xb.ap()[1])
            nc.gpsimd.dma_start(out=sxq[2][:], in_=d_xb.ap()[2])
            nc.sync.dma_start(out=sxq[0][:], in_=d_xb.ap()[0])
            nc.scalar.dma_start(out=sxq[4][:], in_=d_xb.ap()[4])
            nc.gpsimd.dma_start(out=sxq[5][:], in_=d_xb.ap()[5])
            nc.sync.dma_start(out=sxq[3][:], in_=d_xb.ap()[3])
            nc.scalar.dma_start(out=sw1[:, W1CH : 2 * W1CH], in_=d_w1.ap()[:, W1CH : 2 * W1CH])
            nc.gpsimd.dma_start(out=sxq[6][:], in_=d_xb.ap()[6])
            nc.sync.dma_start(out=sw2[:], in_=d_w2.ap())
            nc.scalar.dma_start(out=sw1[:, 2 * W1CH : 3 * W1CH], in_=d_w1.ap()[:, 2 * W1CH : 3 * W1CH])
            nc.gpsimd.dma_start(out=sw1[:, 3 * W1CH : 4 * W1CH], in_=d_w1.ap()[:, 3 * W1CH : 4 * W1CH])
            nc.sync.dma_start(out=sb2[:], in_=d_b2.ap())
            nc.scalar.dma_start(out=sw1[:, 4 * W1CH : 5 * W1CH], in_=d_w1.ap()[:, 4 * W1CH : 5 * W1CH])
            nc.gpsimd.dma_start(out=sw1[:, 5 * W1CH : 6 * W1CH], in_=d_w1.ap()[:, 5 * W1CH : 6 * W1CH])
            nc.sync.dma_start(out=seb[:], in_=d_eb.ap())
            nc.gpsimd.dma_start(out=sv[:], in_=d_v.ap())

            u = singles.tile([128, NSEG * 70], f32)
            r = singles.tile([128, NSEG * 14], f32)

            # two 4-bank psum tiles; judge k uses slot k%2 for L1/L2.
            # Layer-3 psum of a (even,odd) judge pair sits in slot A's
            # banks 2 (even) and 3 (odd) at cols L3COL..L3COL+70.
            pS = [
                pp.tile([128, 3, 512], f32, tag="s0", name="s0"),
                pp.tile([128, 3, 512], f32, tag="s1", name="s1"),
            ]
            pL3 = pp.tile([128, 1, 512], f32, tag="pL3", name="pL3")

            def mm1(k):
                j, n0, C = segs[k]
                P = pS[k % 2]
                w = -(-7 * C // 3)
                for q in range(Q):
                    lo = q * C
                    while lo < (q + 1) * C:
                        b = lo // w
                        hi = min((q + 1) * C, (b + 1) * w)
                        nc.tensor.matmul(
                            out=P[:, b, lo - b * w : hi - b * w],
                            lhsT=sw1[:, (j * Q + q) * H1 : (j * Q + q + 1) * H1],
                            rhs=sxq[q][:, n0 + lo - q * C : n0 + hi - q * C],
                            start=True,
                            stop=True,
                        )
                        lo = hi

            def act1(k):
                j, n0, C = segs[k]
                w = -(-7 * C // 3)
                z1 = zp.tile([128, 8 * CMAXJ], dt2, tag="z1", name="z1")
                nc.scalar.activation(
                    out=z1[:, : 3 * w].rearrange("p (b c) -> p b c", b=3),
                    in_=pS[k % 2][:, :, :w],
                    func=AF.Tanh,
                )
                return z1

            def mm2(k, z1):
                j, n0, C = segs[k]
                w = -(-7 * C // 3)
                for b in range(3):
                    nc.tensor.matmul(
                        out=pS[k % 2][:, b, 0:w],
                        lhsT=sw2[:, j * H2 : (j + 1) * H2],
                        rhs=z1[:, b * w : (b + 1) * w],
                        start=True,
                        stop=True,
                    )

            def act2(k):
                j, n0, C = segs[k]
                w = -(-7 * C // 3)
                z2 = zp.tile([128, 8 * CMAXJ], dt3, tag="z2", name="z2")
                nc.scalar.activation(
                    out=z2[:, : 3 * w].rearrange("p (b c) -> p b c", b=3),
                    in_=pS[k % 2][:, :, :w],
                    func=AF.Tanh,
                    bias=sb2[:, j : j + 1],
                )
                return z2

            def mm3(k, z2):
                j, n0, C = segs[k]
                HC = C // 2
                pos = 140 * ((k // 2) % 3) + 70 * (k % 2)
                for q in range(Q):
                    for c in range(2):
                        nc.tensor.matmul(
                            out=pL3[:, 0, pos + c * 35 + q * O : pos + c * 35 + (q + 1) * O],
                            lhsT=z2[:, q * C + c * HC : q * C + c * HC + 128],
                            rhs=sv[:, (j * Q + q) * O : (j * Q + q + 1) * O],
                            start=True,
                            stop=True,
                        )

            def act3(e, npair):
                # exp over the L3 psum of segments e..e+npair-1
                pos = 140 * ((e // 2) % 3) + 70 * (e % 2)
                nc.scalar.activation(
                    out=u[:, 70 * e : 70 * (e + npair)],
                    in_=pL3[:, 0, pos : pos + 70 * npair],
                    func=AF.Exp,
                )

            def softmax_dma(e, npair):
                span = 70 * npair
                ug = u[:, 70 * e : 70 * e + span]
                nc.vector.tensor_mul(out=ug, in0=ug, in1=seb[:, 70 * e : 70 * e + span])
                ug3 = ug.rearrange("p (t o) -> p t o", o=O)
                rg = r[:, 14 * e : 14 * e + 14 * npair]
                nc.vector.tensor_reduce(out=rg, in_=ug3, axis=AX.X, op=ALU.add)
                nc.vector.reciprocal(out=rg, in_=rg)
                nc.vector.tensor_mul(
                    out=ug3,
                    in0=ug3,
                    in1=rg.unsqueeze(2).broadcast_to((128, 14 * npair, O)),
                )
                for m in range(e, e + npair):
                    j, n0, C = segs[m]
                    HC = C // 2
                    eng = nc.sync if m % 2 == 0 else nc.gpsimd
                    eng.dma_start(
                        out=d_out.ap()[n0 : n0 + C, :].rearrange(
                            "(p c) e -> p c e", c=2
                        ),
                        in_=u[0:HC, 70 * m : 70 * m + 70].rearrange(
                            "p (c e) -> p c e", c=2
                        ),
                    )

            # ---- software-pipelined main loop ----
            z1s = {}
            z2s = {}
            for k in range(NSEG + 1):
                if k < NSEG:
                    mm1(k)
                    z1s[k] = act1(k)
                if k >= 1:
                    mm2(k - 1, z1s.pop(k - 1))
                    z2s[k - 1] = act2(k - 1)
                    mm3(k - 1, z2s.pop(k - 1))
                    if (k - 1) % 2 == 1:
                        act3(k - 2, 2)
                        softmax_dma(k - 2, 2)
                    elif k == NSEG:
                        act3(k - 1, 1)
                        softmax_dma(k - 1, 1)

    nc.compile()
    return nc


def kernel(x, judge_ids, W1, W1_a, W2, W2_a, V, V_a):
    from concourse import bass_utils

    x = np.ascontiguousarray(np.asarray(x), dtype=np.float32)
    jid = np.asarray(judge_ids)
    w1s, w2s, b2s, vs, expb = _fold_weights(
        np.asarray(W1, np.float32),
        np.asarray(W1_a, np.float32),
        np.asarray(W2, np.float32),
        np.asarray(W2_a, np.float32),
        np.asarray(V, np.float32),
        np.asarray(V_a, np.float32),
    )
    core_idx, segs, ncap = _plan(jid)
    expbs = _expand_expb(expb, segs)

    nc = _build_program(ncap, segs)
    in_maps = _pack_inputs(x, core_idx, segs, ncap, w1s, w2s, b2s, vs, expbs)
    res = bass_utils.run_bass_kernel_spmd(nc, in_maps, core_ids=list(range(NCORES)))

    out_full = np.empty((x.shape[0], Q, O), dtype=np.float32)
    for c in range(NCORES):
        out_full[core_idx[c]] = res.results[c]["out"].reshape(ncap, Q, O)
    return out_full
